# revision 22
# baseline (speedup 1.0000x reference)
"""Trainium2 Bass kernel for nn_DSCAMSFF (1x1 conv + per-group CBAM gating).

Only x4 is live in the reference model (cov1-3 / the attention path are dead
code that returns its first argument). Effective computation per batch b:

  a  = conv1x1(x4[b]) : [512, 256]          (w [512,2048], pixels flattened)
  per group g (channels of group g are a[(g%2)*256 : (g%2+1)*256]):
    avg_g = mean_px(a_g)                       [256]
    h_g   = relu(fc1_w[g] @ avg_g + fc1_b[g])  [64]
    ca_g  = sigmoid(fc2_w[g] @ h_g + fc2_b[g]) [256]
    sa_g  = sigmoid((ca_g*sa_w[g]) . a_g + sa_b[g])   [256 px]
    z_g   = sigmoid(a_g * ca_g[:,None] * sa_g[None,:])
    mask  = where(z_g > mean(z_g), 1, z_g)
    out_g = a_g * (mask + 1)

Sharding: pure data-parallel over batch (8 cores x 1 batch element),
parameters replicated.

v2 schedule: input DMAs issued first (x, w0, w1, params, w2, w3) so conv
m-tiles start as weights land; the pixel-mean is folded into the conv as a
257th x column; output is written fp16 and upcast on host; the z-chain is
balanced across ACT (sigmoid w/ per-partition ca scale), DVE (u-mult, group
z-sums, fused mask-mul) and GpSimd (one u-mult per half).
"""

import numpy as np

N_CORES = 8
P = 128
PX = 256            # 16*16 pixels
PXA = 257           # pixels + folded mean column
KT = 16             # 2048 / 128 K tiles
MT = 4              # 512 / 128 conv out tiles

# s16 packed layout (columns)
_W1_OFF = 0         # [p, kt, m]    2*2*256  = 1024
_W2_OFF = 1024      # [p, i, s, m]  2*4*2*128 = 2048
_B2_OFF = 3072      # fc2 bias pack [8 rows, p*128 cols] = 256
_NS16 = 3328
# r16 row tensor (partition 0 only)
_CB_OFF = 0         # conv bias row [512]
_B1_OFF = 512       # fc1 bias row  [p, mt, 128] = 512
_SAB_OFF = 1024     # spatial bias row [8, 128] (replicated)
_NR16 = 2048
# s32 packed fp32
_SAW_OFF = 0        # [p, s, i] 16
_NS32 = 16

_CACHE = {}


def _register_dve_ops():
    """Register the fused mask-mul DVE op (idempotent, runtime-only)."""
    from concourse import dve_ops as DO
    from concourse.dve_spec import Spec, Src0, Src1, One, select

    if "DSCAM_MASK_MUL" in DO._SUB_OPCODE_FOR_NAME:
        return next(o for o in DO.OPS if o.name == "DSCAM_MASK_MUL")

    from concourse.dve_spec import C0, lower
    from concourse.dve_uop import DveOpSpec

    name = "DSCAM_MASK_MUL"
    row = DO._CUSTOM_DVE_ROW_BASE + len(DO.OPS)
    DO._SUB_OPCODE_FOR_NAME[name] = row
    spec = Spec(
        body=Src1 * (One + select(Src0 > C0, One, Src0)),
        reference=lambda in0, in1, s0, s1, imm2:
            (in1.astype(np.float32)
             * (1.0 + np.where(in0.astype(np.float32) > s0, 1.0,
                               in0.astype(np.float32)))).astype(np.float32),
    )
    shas = {}
    for ver in ("v3", "v4"):
        try:
            uops = lower(spec, ver=ver)
            shas[ver] = DveOpSpec(name=name, opcode=row, uops=uops,
                                  rd1_en=True).sha(ver)
        except Exception:
            pass
    op = DO.DveOp(name, spec, subdim=False, uops_sha=shas)
    DO.OPS.append(op)
    DO.CUSTOM_DVE_SPECS[name] = spec
    return op


def _build_program():
    import concourse.mybir as mybir
    import concourse.tile as tile
    from concourse import bacc

    fp32 = mybir.dt.float32
    fp16 = mybir.dt.float16
    Act = mybir.ActivationFunctionType
    Alu = mybir.AluOpType
    AX = mybir.AxisListType

    _MSK_OP = _register_dve_ops()

    nc = bacc.Bacc("TRN2", target_bir_lowering=False, debug=False)

    x_d = nc.dram_tensor("x", [P, KT, PXA], fp16, kind="ExternalInput").ap()
    w_d = nc.dram_tensor("w", [MT, P, KT, P], fp16, kind="ExternalInput").ap()
    s16_d = nc.dram_tensor("s16", [P, _NS16], fp16, kind="ExternalInput").ap()
    r16_d = nc.dram_tensor("r16", [1, _NR16], fp16, kind="ExternalInput").ap()
    s32_d = nc.dram_tensor("s32", [P, _NS32], fp32, kind="ExternalInput").ap()
    i8_d = nc.dram_tensor("i8", [8, 8], fp16, kind="ExternalInput").ap()
    # out chunk c = 2*p + pair holds groups (p + 2*(2*pair), p + 2*(2*pair+1))
    out_d = nc.dram_tensor("out", [4, P, 2, 2, PX], fp16,
                           kind="ExternalOutput").ap()

    with tile.TileContext(nc) as tc:
        with (
            tc.tile_pool(name="singles", bufs=1) as singles,
            tc.tile_pool(name="upool", bufs=4) as upool,
            tc.tile_pool(name="zpool", bufs=5) as zpool,
            tc.tile_pool(name="otp", bufs=3) as otp,
            tc.tile_pool(name="psC", bufs=2, space="PSUM") as psC,
            tc.tile_pool(name="psS", bufs=2, space="PSUM") as psS,
            tc.tile_pool(name="psT", bufs=2, space="PSUM") as psT,
            tc.tile_pool(name="psW", bufs=2, space="PSUM") as psW,
        ):
            # ---- input DMAs first: ordered by need time ----
            xt = singles.tile([P, KT, PXA], fp16, tag="xt")
            wt = [singles.tile([P, KT, P], fp16, tag=f"w{m}", name=f"w{m}")
                  for m in range(MT)]
            s16 = singles.tile([P, _NS16], fp16, tag="s16")
            r16 = singles.tile([1, _NR16], fp16, tag="r16")
            s32 = singles.tile([P, _NS32], fp32, tag="s32")

            nc.sync.dma_start(out=xt, in_=x_d)
            nc.sync.dma_start(out=wt[0], in_=w_d[0])
            nc.sync.dma_start(out=wt[1], in_=w_d[1])
            nc.sync.dma_start(out=s16, in_=s16_d)
            nc.sync.dma_start(out=r16, in_=r16_d)
            nc.sync.dma_start(out=s32, in_=s32_d)
            id8 = singles.tile([8, 8], fp16, tag="id8")
            nc.sync.dma_start(out=id8, in_=i8_d)
            nc.sync.dma_start(out=wt[2], in_=w_d[2])
            nc.sync.dma_start(out=wt[3], in_=w_d[3])

            # parameter views
            w1v = s16[:, _W1_OFF:_W1_OFF + 1024].rearrange(
                "P (p k m) -> P p k m", p=2, k=2)
            w2v = s16[:, _W2_OFF:_W2_OFF + 2048].rearrange(
                "P (p i s m) -> P p i s m", p=2, i=4, s=2)
            b2v = s16[0:8, _B2_OFF:_B2_OFF + 256].rearrange(
                "o (p m) -> o p m", p=2)
            cbias = r16[:, _CB_OFF:_CB_OFF + 512].rearrange(
                "o (m c) -> o m c", m=4)
            b1row = r16[:, _B1_OFF:_B1_OFF + 512].rearrange(
                "o (p t c) -> o p t c", p=2, t=2)
            sab16 = r16[:, _SAB_OFF:_SAB_OFF + 1024].rearrange(
                "o (g c) -> o g c", g=8)
            sawv = s32[:, _SAW_OFF:_SAW_OFF + 16].rearrange(
                "P (p s i) -> P p s i", p=2, s=2)

            # constants (memsets overlap the DMA stream)
            ones16 = singles.tile([1, PXA], fp16, tag="ones16")
            nc.gpsimd.memset(ones16, 1.0)
            one32 = singles.tile([1, 1], fp32, tag="one32")
            nc.gpsimd.memset(one32, 1.0)
            onesPK = singles.tile([P, PX], fp16, tag="onesPK")
            nc.gpsimd.memset(onesPK, 1.0)
            # ACT table preload; PE HAM warmup while input DMAs stream
            tl = singles.tile([1, 1], fp32, tag="tl")
            nc.scalar.activation(out=tl, in_=ones16[:, 0:1], func=Act.Sigmoid)
            for wu in range(18):
                wps = psW.tile([P, PX], fp32, tag="wps")
                nc.tensor.matmul(wps, lhsT=onesPK[:, 0:P], rhs=onesPK,
                                 start=True, stop=True)

            a16 = [None, None]    # per half: [P, 2, PXA] fp16 (col 256 = avg)
            h16 = [None, None]
            ca = [None, None]
            weff16 = [None, None]

            def conv_m(m):
                # one conv out tile -> psum [P, PXA]; col 256 = pixel mean
                ps = psC.tile([P, PXA], fp32, tag="conv")
                for kt in range(KT):
                    nc.tensor.matmul(ps, lhsT=wt[m][:, kt, :],
                                     rhs=xt[:, kt, :],
                                     start=(kt == 0), stop=False)
                nc.tensor.matmul(ps, lhsT=cbias[:, m, :], rhs=ones16,
                                 start=False, stop=True)
                p, s = m // 2, m % 2
                if a16[p] is None:
                    a16[p] = singles.tile([P, 2, PXA], fp16, tag=f"a16_{p}",
                                          name=f"a16_{p}")
                # PSUM -> SBUF fp16 on the ACT engine (idle during conv)
                nc.scalar.copy(out=a16[p][:, s, :], in_=ps)

            def fc_chain(p):
                # fc1: h = relu(W1 @ avg + b1); avg = a16 col 256
                hp = psT.tile([P, 2], fp32, tag="tiny")
                for mt in (0, 1):
                    for kt in (0, 1):
                        nc.tensor.matmul(
                            hp[:, mt:mt + 1],
                            lhsT=w1v[:, p, kt, mt * P:(mt + 1) * P],
                            rhs=a16[p][:, kt, 256:257],
                            start=(kt == 0), stop=False)
                    nc.tensor.matmul(
                        hp[:, mt:mt + 1], lhsT=b1row[:, p, mt, :],
                        rhs=ones16[:, 0:1], start=False, stop=True)
                h16[p] = singles.tile([P, 2], fp16, tag=f"h{p}", name=f"h{p}")
                nc.scalar.activation(out=h16[p], in_=hp, func=Act.Relu)
                # fc2: ca = sigmoid(W2 @ h + b2). W2's unused 64 rows are
                # zero, so the stacked h (both groups of an mt) is safe as rhs.
                cp = psT.tile([P, 2, 4], fp32, tag="tiny")
                for s in (0, 1):
                    for i in range(4):
                        nc.tensor.matmul(
                            cp[:, s, i:i + 1], lhsT=w2v[:, p, i, s, :],
                            rhs=h16[p][:, i // 2:i // 2 + 1],
                            start=True, stop=False)
                        nc.tensor.matmul(
                            cp[:, s, i:i + 1], lhsT=b2v[:, p, :],
                            rhs=id8[:, 4 * s + i:4 * s + i + 1],
                            start=False, stop=True)
                ca[p] = singles.tile([P, 2, 4], fp32, tag=f"ca{p}",
                                     name=f"ca{p}")
                nc.scalar.activation(out=ca[p], in_=cp, func=Act.Sigmoid)
                weff16[p] = singles.tile([P, 2, 4], fp16, tag=f"we{p}",
                                         name=f"we{p}")
                nc.vector.tensor_tensor(out=weff16[p], in0=ca[p],
                                        in1=sawv[:, p], op=Alu.mult)

            def saz_mask(p):
                av = a16[p][:, :, 0:PX]
                sarep = [None, None]
                for j in range(2):
                    # groups g = p + 2*(2j), p + 2*(2j+1) share one psum bank
                    sps = psS.tile([P, 2, PX], fp32, tag="sa")
                    for jj in (0, 1):
                        i = 2 * j + jj
                        for s in (0, 1):
                            nc.tensor.matmul(
                                sps[:, jj, :],
                                lhsT=weff16[p][:, s, i:i + 1]
                                .to_broadcast((P, P)),
                                rhs=av[:, s, :],
                                start=(s == 0), stop=False)
                        nc.tensor.matmul(
                            sps[:, jj, :], lhsT=sab16[:, p + 2 * i, :],
                            rhs=ones16[:, 0:PX], start=False, stop=True)
                    sarep[j] = upool.tile([P, 2, PX], fp16, tag="sarep",
                                          name="sarep")
                    nc.scalar.activation(out=sarep[j], in_=sps,
                                         func=Act.Sigmoid)
                zsum = singles.tile([P, 2], fp16, tag=f"zs{p}", name=f"zs{p}")
                tots = [None, None]
                zs = [None] * 4
                for i in range(4):
                    sr = sarep[i // 2][:, i % 2, :]
                    u16 = upool.tile([P, 2, PX], fp16, tag="u16")
                    # engine balance: DVE does 2 u-mults, GpSimd the other 2
                    eng = nc.gpsimd if i >= 2 else nc.vector
                    eng.tensor_tensor(
                        out=u16, in0=av,
                        in1=sr[:, None, :].to_broadcast((P, 2, PX)),
                        op=Alu.mult)
                    z16 = zpool.tile([P, 2, PX], fp16, tag="z")
                    for s in (0, 1):
                        nc.scalar.activation(
                            out=z16[:, s, :], in_=u16[:, s, :],
                            func=Act.Sigmoid, scale=ca[p][:, s, i:i + 1])
                    zs[i] = z16
                    if i < 2:
                        # per-partition z-sums on DVE, cross-partition via MM
                        with nc.allow_low_precision(
                                reason="fp16 partial z-sums; ~1e-4 on mean"):
                            nc.vector.tensor_reduce(
                                zsum[:, i:i + 1],
                                z16.rearrange("P s f -> P (s f)"),
                                axis=AX.X, op=Alu.add)
                    else:
                        # full-group sum in one GpSimd reduce (fp32 scalar)
                        tots[i - 2] = singles.tile(
                            [1, 1], fp32, tag=f"tot{p}{i}", name=f"tot{p}{i}")
                        nc.gpsimd.tensor_reduce(
                            tots[i - 2], z16, axis=AX.XYZWC, op=Alu.add)
                    if i == 1:
                        # pair-0 mean: cross-partition sum via all-ones matmul
                        zr = psT.tile([P, 2], fp32, tag="tiny")
                        nc.tensor.matmul(zr, lhsT=onesPK[:, 0:P],
                                         rhs=zsum, start=True, stop=True)
                        pm = singles.tile([P, 2], fp32, tag=f"pm{p}0",
                                          name=f"pm{p}0")
                        nc.vector.tensor_scalar_mul(pm, zr, 1.0 / 65536.0)
                        ot = otp.tile([P, 2, 2, PX], fp16, tag="ot", name="ot")
                        for ii in (0, 1):
                            # fused out = a * (1 + where(z > mean, 1, z))
                            nc.vector._custom_dve(
                                _MSK_OP, out=ot[:, ii], in0=zs[ii],
                                in1=av, s0=pm[:, ii:ii + 1])
                        nc.sync.dma_start(out=out_d[2 * p], in_=ot)
                    elif i == 3:
                        # pair-1 mean: broadcast the scalar totals via K=1 MMs
                        zr = psT.tile([P, 2], fp32, tag="tiny")
                        for ii in (0, 1):
                            nc.tensor.matmul(
                                zr[:, ii:ii + 1],
                                lhsT=tots[ii].to_broadcast((1, P)),
                                rhs=one32, start=True, stop=True)
                        pm = singles.tile([P, 2], fp32, tag=f"pm{p}1",
                                          name=f"pm{p}1")
                        nc.vector.tensor_scalar_mul(pm, zr, 1.0 / 65536.0)
                        ot = otp.tile([P, 2, 2, PX], fp16, tag="ot", name="ot")
                        for ii in (0, 1):
                            nc.vector._custom_dve(
                                _MSK_OP, out=ot[:, ii], in0=zs[2 + ii],
                                in1=av, s0=pm[:, ii:ii + 1])
                        nc.sync.dma_start(out=out_d[2 * p + 1], in_=ot)

            conv_m(0)
            conv_m(1)
            fc_chain(0)
            saz_mask(0)
            conv_m(2)
            conv_m(3)
            fc_chain(1)
            saz_mask(1)

    nc.finalize()
    return nc


def _prep_core_inputs(x4b, w, s16, r16, s32):
    x = np.ascontiguousarray(
        x4b.reshape(KT, P, PX).transpose(1, 0, 2))
    xa = np.empty((P, KT, PXA), np.float16)
    xa[:, :, 0:PX] = x.astype(np.float16)
    xa[:, :, PX] = (x.mean(axis=2) * 1.0).astype(np.float16)
    return {"x": xa, "w": w, "s16": s16, "r16": r16, "s32": s32,
            "i8": np.eye(8, dtype=np.float16)}


def _prep_params(cov4_w, cov4_b, fc1_w, fc1_b, fc2_w, fc2_b, sa_w, sa_b):
    f32 = np.float32
    w2d = np.asarray(cov4_w, f32).reshape(512, 2048)
    wr = w2d.reshape(MT, P, KT, P)                 # [m, mc, kt, part]
    w_arr = np.ascontiguousarray(wr.transpose(0, 3, 2, 1)).astype(np.float16)

    fc1_w = np.asarray(fc1_w, f32)
    fc1_b = np.asarray(fc1_b, f32)
    fc2_w = np.asarray(fc2_w, f32)
    fc2_b = np.asarray(fc2_b, f32)
    sa_w = np.asarray(sa_w, f32)
    sa_b = np.asarray(sa_b, f32)

    w1 = np.zeros((P, 2, 2, 256), f32)
    w2 = np.zeros((P, 2, 4, 2, P), f32)
    b2 = np.zeros((8, 2, P), f32)
    b1 = np.zeros((2, 2, P), f32)
    saw = np.zeros((P, 2, 2, 4), f32)
    for p in range(2):
        W1s = np.concatenate([fc1_w[p + 2 * i] for i in range(4)], axis=0)
        b1s = np.concatenate([fc1_b[p + 2 * i] for i in range(4)], axis=0)
        for kt in range(2):
            w1[:, p, kt, :] = W1s[:, kt * P:(kt + 1) * P].T
        b1[p, 0] = b1s[:P]
        b1[p, 1] = b1s[P:]
        for i in range(4):
            g = p + 2 * i
            lo = 64 * (i % 2)           # rows holding fc2 weights
            for s in range(2):
                w2[lo:lo + 64, p, i, s, :] = fc2_w[g][s * P:(s + 1) * P, :].T
                b2[4 * s + i, p, :] = fc2_b[g, s * P:(s + 1) * P]
                saw[:, p, s, i] = sa_w[g, s * P:(s + 1) * P]

    s16 = np.zeros((P, _NS16), np.float16)
    s16[:, _W1_OFF:_W1_OFF + 1024] = w1.reshape(P, 1024).astype(np.float16)
    s16[:, _W2_OFF:_W2_OFF + 2048] = w2.reshape(P, 2048).astype(np.float16)
    s16[0:8, _B2_OFF:_B2_OFF + 256] = b2.reshape(8, 256).astype(np.float16)

    r16 = np.zeros((1, _NR16), np.float16)
    r16[0, _CB_OFF:_CB_OFF + 512] = np.asarray(cov4_b, f32).astype(np.float16)
    r16[0, _B1_OFF:_B1_OFF + 512] = b1.reshape(512).astype(np.float16)
    r16[0, _SAB_OFF:_SAB_OFF + 1024] = np.repeat(
        np.asarray(sa_b, f32), 128).astype(np.float16)

    s32 = np.zeros((P, _NS32), f32)
    s32[:, _SAW_OFF:_SAW_OFF + 16] = saw.reshape(P, 16)
    return w_arr, s16, r16, s32


def kernel(**inputs):
    from concourse.bass_utils import run_bass_kernel_spmd

    if "nc" not in _CACHE:
        _CACHE["nc"] = _build_program()
    nc = _CACHE["nc"]

    x4 = np.asarray(inputs["x4"], np.float32)
    B = x4.shape[0]
    w_arr, s16, r16, s32 = _prep_params(
        inputs["cov4_w"], inputs["cov4_b"],
        inputs["gce_fc1_w"], inputs["gce_fc1_b"],
        inputs["gce_fc2_w"], inputs["gce_fc2_b"],
        inputs["gce_sa_w"], inputs["gce_sa_b"])

    in_maps = [
        _prep_core_inputs(x4[b].reshape(2048, PX), w_arr, s16, r16, s32)
        for b in range(B)
    ]
    res = run_bass_kernel_spmd(nc, in_maps, list(range(N_CORES)))
    _CACHE["last_results"] = res

    out = np.empty((B, 2048, 16, 16), np.float32)
    for b in range(B):
        o = res.results[b]["out"].astype(np.float32)   # [4, 128, 2, 2, 256]
        full = out[b].reshape(8, 2, P, PX)             # [g, s, part, px]
        for p in range(2):
            for pair in range(2):
                for ii in range(2):
                    g = p + 2 * (2 * pair + ii)
                    full[g] = o[2 * p + pair, :, ii, :, :].transpose(1, 0, 2)
    return out


# revision 26
# speedup vs baseline: 1.0362x; 1.0362x over previous
"""Trainium2 Bass kernel for nn_DSCAMSFF (1x1 conv + per-group CBAM gating).

Only x4 is live in the reference model (cov1-3 / the attention path are dead
code that returns its first argument). Effective computation per batch b:

  a  = conv1x1(x4[b]) : [512, 256]          (w [512,2048], pixels flattened)
  per group g (channels of group g are a[(g%2)*256 : (g%2+1)*256]):
    avg_g = mean_px(a_g)                       [256]
    h_g   = relu(fc1_w[g] @ avg_g + fc1_b[g])  [64]
    ca_g  = sigmoid(fc2_w[g] @ h_g + fc2_b[g]) [256]
    sa_g  = sigmoid((ca_g*sa_w[g]) . a_g + sa_b[g])   [256 px]
    z_g   = sigmoid(a_g * ca_g[:,None] * sa_g[None,:])
    mask  = where(z_g > mean(z_g), 1, z_g)
    out_g = a_g * (mask + 1)

Sharding: pure data-parallel over batch (8 cores x 1 batch element),
parameters replicated.

v2 schedule: input DMAs issued first (x, w0, w1, params, w2, w3) so conv
m-tiles start as weights land; the pixel-mean is folded into the conv as a
257th x column; output is written fp16 and upcast on host; the z-chain is
balanced across ACT (sigmoid w/ per-partition ca scale), DVE (u-mult, group
z-sums, fused mask-mul) and GpSimd (one u-mult per half).
"""

import numpy as np

N_CORES = 8
P = 128
PX = 256            # 16*16 pixels
PXA = 257           # pixels + folded mean column
KT = 16             # 2048 / 128 K tiles
MT = 4              # 512 / 128 conv out tiles

# s16 packed layout (columns)
_W1_OFF = 0         # [p, kt, m]    2*2*256  = 1024
_W2_OFF = 1024      # [p, i, s, m]  2*4*2*128 = 2048
_B2_OFF = 3072      # fc2 bias pack [8 rows, p*128 cols] = 256
_SAW_OFF = 3328     # [p, s, i] 16
_NS16 = 3344
# r16 row tensor (partition 0 only)
_CB_OFF = 0         # conv bias row [512]
_B1_OFF = 512       # fc1 bias row  [p, mt, 128] = 512
_SAB_OFF = 1024     # spatial bias row [8, 128] (replicated)
_NR16 = 2048

_CACHE = {}


def _register_dve_ops():
    """Register the fused mask-mul DVE op (idempotent, runtime-only)."""
    from concourse import dve_ops as DO
    from concourse.dve_spec import Spec, Src0, Src1, One, select

    if "DSCAM_MASK_MUL" in DO._SUB_OPCODE_FOR_NAME:
        return next(o for o in DO.OPS if o.name == "DSCAM_MASK_MUL")

    from concourse.dve_spec import C0, lower
    from concourse.dve_uop import DveOpSpec

    name = "DSCAM_MASK_MUL"
    row = DO._CUSTOM_DVE_ROW_BASE + len(DO.OPS)
    DO._SUB_OPCODE_FOR_NAME[name] = row
    spec = Spec(
        body=Src1 * (One + select(Src0 > C0, One, Src0)),
        reference=lambda in0, in1, s0, s1, imm2:
            (in1.astype(np.float32)
             * (1.0 + np.where(in0.astype(np.float32) > s0, 1.0,
                               in0.astype(np.float32)))).astype(np.float32),
    )
    shas = {}
    for ver in ("v3", "v4"):
        try:
            uops = lower(spec, ver=ver)
            shas[ver] = DveOpSpec(name=name, opcode=row, uops=uops,
                                  rd1_en=True).sha(ver)
        except Exception:
            pass
    op = DO.DveOp(name, spec, subdim=False, uops_sha=shas)
    DO.OPS.append(op)
    DO.CUSTOM_DVE_SPECS[name] = spec
    return op


def _build_program():
    import concourse.mybir as mybir
    import concourse.tile as tile
    from concourse import bacc

    fp32 = mybir.dt.float32
    fp16 = mybir.dt.float16
    Act = mybir.ActivationFunctionType
    Alu = mybir.AluOpType
    AX = mybir.AxisListType

    _MSK_OP = _register_dve_ops()

    nc = bacc.Bacc("TRN2", target_bir_lowering=False, debug=False)

    x_d = nc.dram_tensor("x", [P, KT, PXA], fp16, kind="ExternalInput").ap()
    w_d = nc.dram_tensor("w", [MT, P, KT, P], fp16, kind="ExternalInput").ap()
    s16_d = nc.dram_tensor("s16", [P, _NS16], fp16, kind="ExternalInput").ap()
    r16_d = nc.dram_tensor("r16", [1, _NR16], fp16, kind="ExternalInput").ap()
    i8_d = nc.dram_tensor("i8", [8, 8], fp16, kind="ExternalInput").ap()
    # out chunk c = 2*p + pair holds groups (p + 2*(2*pair), p + 2*(2*pair+1))
    out_d = nc.dram_tensor("out", [4, P, 2, 2, PX], fp16,
                           kind="ExternalOutput").ap()

    with tile.TileContext(nc) as tc:
        with (
            tc.tile_pool(name="singles", bufs=1) as singles,
            tc.tile_pool(name="upool", bufs=4) as upool,
            tc.tile_pool(name="zpool", bufs=5) as zpool,
            tc.tile_pool(name="otp", bufs=3) as otp,
            tc.tile_pool(name="psC", bufs=2, space="PSUM") as psC,
            tc.tile_pool(name="psS", bufs=2, space="PSUM") as psS,
            tc.tile_pool(name="psT", bufs=2, space="PSUM") as psT,
            tc.tile_pool(name="psW", bufs=2, space="PSUM") as psW,
        ):
            # ---- input DMAs first: ordered by need time ----
            xt = singles.tile([P, KT, PXA], fp16, tag="xt")
            wt = [singles.tile([P, KT, P], fp16, tag=f"w{m}", name=f"w{m}")
                  for m in range(MT)]
            s16 = singles.tile([P, _NS16], fp16, tag="s16")
            r16 = singles.tile([1, _NR16], fp16, tag="r16")
            id8 = singles.tile([8, 8], fp16, tag="id8")

            # x/w stream on the sync HWDGE ring; params on the scalar ring
            nc.sync.dma_start(out=xt, in_=x_d)
            nc.sync.dma_start(out=wt[0], in_=w_d[0])
            nc.sync.dma_start(out=wt[1], in_=w_d[1])
            nc.sync.dma_start(out=wt[2], in_=w_d[2])
            nc.sync.dma_start(out=wt[3], in_=w_d[3])
            nc.scalar.dma_start(out=s16, in_=s16_d)
            nc.scalar.dma_start(out=r16, in_=r16_d)
            nc.scalar.dma_start(out=id8, in_=i8_d)

            # parameter views
            w1v = s16[:, _W1_OFF:_W1_OFF + 1024].rearrange(
                "P (p k m) -> P p k m", p=2, k=2)
            w2v = s16[:, _W2_OFF:_W2_OFF + 2048].rearrange(
                "P (p i s m) -> P p i s m", p=2, i=4, s=2)
            b2v = s16[0:8, _B2_OFF:_B2_OFF + 256].rearrange(
                "o (p m) -> o p m", p=2)
            cbias = r16[:, _CB_OFF:_CB_OFF + 512].rearrange(
                "o (m c) -> o m c", m=4)
            b1row = r16[:, _B1_OFF:_B1_OFF + 512].rearrange(
                "o (p t c) -> o p t c", p=2, t=2)
            sab16 = r16[:, _SAB_OFF:_SAB_OFF + 1024].rearrange(
                "o (g c) -> o g c", g=8)
            sawv = s16[:, _SAW_OFF:_SAW_OFF + 16].rearrange(
                "P (p s i) -> P p s i", p=2, s=2)

            # constants (memsets overlap the DMA stream)
            ones16 = singles.tile([1, PXA], fp16, tag="ones16")
            nc.gpsimd.memset(ones16, 1.0)
            onesPK = singles.tile([P, PX], fp16, tag="onesPK")
            nc.gpsimd.memset(onesPK, 1.0)
            # ACT table preload; PE HAM warmup while input DMAs stream
            tl = singles.tile([1, 1], fp32, tag="tl")
            nc.scalar.activation(out=tl, in_=ones16[:, 0:1], func=Act.Sigmoid)
            for wu in range(18):
                wps = psW.tile([P, PX], fp32, tag="wps")
                nc.tensor.matmul(wps, lhsT=onesPK[:, 0:P], rhs=onesPK,
                                 start=True, stop=True)

            a16 = [None, None]    # per half: [P, 2, PXA] fp16 (col 256 = avg)
            h16 = [None, None]
            ca = [None, None]
            weff16 = [None, None]

            def conv_m(m):
                # one conv out tile -> psum [P, PXA]; col 256 = pixel mean
                ps = psC.tile([P, PXA], fp32, tag="conv")
                for kt in range(KT):
                    nc.tensor.matmul(ps, lhsT=wt[m][:, kt, :],
                                     rhs=xt[:, kt, :],
                                     start=(kt == 0), stop=False)
                nc.tensor.matmul(ps, lhsT=cbias[:, m, :], rhs=ones16,
                                 start=False, stop=True)
                p, s = m // 2, m % 2
                if a16[p] is None:
                    a16[p] = singles.tile([P, 2, PXA], fp16, tag=f"a16_{p}",
                                          name=f"a16_{p}")
                # PSUM -> SBUF fp16 on the ACT engine (idle during conv)
                nc.scalar.copy(out=a16[p][:, s, :], in_=ps)

            def fc_chain(p):
                # fc1: h = relu(W1 @ avg + b1); avg = a16 col 256
                hp = psT.tile([P, 2], fp32, tag="tiny")
                for mt in (0, 1):
                    for kt in (0, 1):
                        nc.tensor.matmul(
                            hp[:, mt:mt + 1],
                            lhsT=w1v[:, p, kt, mt * P:(mt + 1) * P],
                            rhs=a16[p][:, kt, 256:257],
                            start=(kt == 0), stop=False)
                    nc.tensor.matmul(
                        hp[:, mt:mt + 1], lhsT=b1row[:, p, mt, :],
                        rhs=ones16[:, 0:1], start=False, stop=True)
                h16[p] = singles.tile([P, 2], fp16, tag=f"h{p}", name=f"h{p}")
                nc.scalar.activation(out=h16[p], in_=hp, func=Act.Relu)
                # fc2: ca = sigmoid(W2 @ h + b2). W2's unused 64 rows are
                # zero, so the stacked h (both groups of an mt) is safe as rhs.
                cp = psT.tile([P, 2, 4], fp32, tag="tiny")
                for s in (0, 1):
                    for i in range(4):
                        nc.tensor.matmul(
                            cp[:, s, i:i + 1], lhsT=w2v[:, p, i, s, :],
                            rhs=h16[p][:, i // 2:i // 2 + 1],
                            start=True, stop=False)
                        nc.tensor.matmul(
                            cp[:, s, i:i + 1], lhsT=b2v[:, p, :],
                            rhs=id8[:, 4 * s + i:4 * s + i + 1],
                            start=False, stop=True)
                ca[p] = singles.tile([P, 2, 4], fp32, tag=f"ca{p}",
                                     name=f"ca{p}")
                nc.scalar.activation(out=ca[p], in_=cp, func=Act.Sigmoid)
                weff16[p] = singles.tile([P, 2, 4], fp16, tag=f"we{p}",
                                         name=f"we{p}")
                nc.vector.tensor_tensor(out=weff16[p], in0=ca[p],
                                        in1=sawv[:, p], op=Alu.mult)

            def saz_mask(p):
                av = a16[p][:, :, 0:PX]
                sarep = [None, None]
                for j in range(2):
                    # groups g = p + 2*(2j), p + 2*(2j+1) share one psum bank
                    sps = psS.tile([P, 2, PX], fp32, tag="sa")
                    for jj in (0, 1):
                        i = 2 * j + jj
                        for s in (0, 1):
                            nc.tensor.matmul(
                                sps[:, jj, :],
                                lhsT=weff16[p][:, s, i:i + 1]
                                .to_broadcast((P, P)),
                                rhs=av[:, s, :],
                                start=(s == 0), stop=False)
                        nc.tensor.matmul(
                            sps[:, jj, :], lhsT=sab16[:, p + 2 * i, :],
                            rhs=ones16[:, 0:PX], start=False, stop=True)
                    sarep[j] = upool.tile([P, 2, PX], fp16, tag="sarep",
                                          name="sarep")
                    nc.scalar.activation(out=sarep[j], in_=sps,
                                         func=Act.Sigmoid)
                zsum = singles.tile([P, 4], fp16, tag=f"zs{p}", name=f"zs{p}")
                zs = [None, None]
                for i in range(4):
                    sr = sarep[i // 2][:, i % 2, :]
                    u16 = upool.tile([P, 2, PX], fp16, tag="u16")
                    # engine balance: DVE does 2 u-mults, GpSimd the other 2
                    eng = nc.gpsimd if i >= 2 else nc.vector
                    eng.tensor_tensor(
                        out=u16, in0=av,
                        in1=sr[:, None, :].to_broadcast((P, 2, PX)),
                        op=Alu.mult)
                    pair, ii = i // 2, i % 2
                    if ii == 0:
                        zs[pair] = zpool.tile([P, 2, 2, PX], fp16, tag="z",
                                              name="zpair")
                    z16 = zs[pair][:, ii]
                    for s in (0, 1):
                        nc.scalar.activation(
                            out=z16[:, s, :], in_=u16[:, s, :],
                            func=Act.Sigmoid, scale=ca[p][:, s, i:i + 1])
                    if ii == 1:
                        # one pair-wide per-partition z-sum on DVE, then the
                        # cross-partition sum via an all-ones fp16 matmul
                        with nc.allow_low_precision(
                                reason="fp16 partial z-sums; ~1e-4 on mean"):
                            nc.vector.tensor_reduce(
                                zsum[:, 2 * pair:2 * pair + 2],
                                zs[pair].rearrange("P i s f -> P i (s f)"),
                                axis=AX.X, op=Alu.add)
                        zr = psT.tile([P, 2], fp32, tag="tiny")
                        nc.tensor.matmul(zr, lhsT=onesPK[:, 0:P],
                                         rhs=zsum[:, 2 * pair:2 * pair + 2],
                                         start=True, stop=True)
                        pm = singles.tile([P, 2], fp32, tag=f"pm{p}{pair}",
                                          name=f"pm{p}{pair}")
                        nc.vector.tensor_scalar_mul(pm, zr, 1.0 / 65536.0)
                        ot = otp.tile([P, 2, 2, PX], fp16, tag="ot", name="ot")
                        for jj in (0, 1):
                            # fused out = a * (1 + where(z > mean, 1, z))
                            nc.vector._custom_dve(
                                _MSK_OP, out=ot[:, jj], in0=zs[pair][:, jj],
                                in1=av, s0=pm[:, jj:jj + 1])
                        nc.sync.dma_start(out=out_d[2 * p + pair], in_=ot)

            conv_m(0)
            conv_m(1)
            fc_chain(0)
            saz_mask(0)
            conv_m(2)
            conv_m(3)
            fc_chain(1)
            saz_mask(1)

    nc.finalize()
    return nc


def _prep_core_inputs(x4b, w, s16, r16):
    x = np.ascontiguousarray(
        x4b.reshape(KT, P, PX).transpose(1, 0, 2))
    xa = np.empty((P, KT, PXA), np.float16)
    xa[:, :, 0:PX] = x.astype(np.float16)
    xa[:, :, PX] = (x.mean(axis=2) * 1.0).astype(np.float16)
    return {"x": xa, "w": w, "s16": s16, "r16": r16,
            "i8": np.eye(8, dtype=np.float16)}


def _prep_params(cov4_w, cov4_b, fc1_w, fc1_b, fc2_w, fc2_b, sa_w, sa_b):
    f32 = np.float32
    w2d = np.asarray(cov4_w, f32).reshape(512, 2048)
    wr = w2d.reshape(MT, P, KT, P)                 # [m, mc, kt, part]
    w_arr = np.ascontiguousarray(wr.transpose(0, 3, 2, 1)).astype(np.float16)

    fc1_w = np.asarray(fc1_w, f32)
    fc1_b = np.asarray(fc1_b, f32)
    fc2_w = np.asarray(fc2_w, f32)
    fc2_b = np.asarray(fc2_b, f32)
    sa_w = np.asarray(sa_w, f32)
    sa_b = np.asarray(sa_b, f32)

    w1 = np.zeros((P, 2, 2, 256), f32)
    w2 = np.zeros((P, 2, 4, 2, P), f32)
    b2 = np.zeros((8, 2, P), f32)
    b1 = np.zeros((2, 2, P), f32)
    saw = np.zeros((P, 2, 2, 4), f32)
    for p in range(2):
        W1s = np.concatenate([fc1_w[p + 2 * i] for i in range(4)], axis=0)
        b1s = np.concatenate([fc1_b[p + 2 * i] for i in range(4)], axis=0)
        for kt in range(2):
            w1[:, p, kt, :] = W1s[:, kt * P:(kt + 1) * P].T
        b1[p, 0] = b1s[:P]
        b1[p, 1] = b1s[P:]
        for i in range(4):
            g = p + 2 * i
            lo = 64 * (i % 2)           # rows holding fc2 weights
            for s in range(2):
                w2[lo:lo + 64, p, i, s, :] = fc2_w[g][s * P:(s + 1) * P, :].T
                b2[4 * s + i, p, :] = fc2_b[g, s * P:(s + 1) * P]
                saw[:, p, s, i] = sa_w[g, s * P:(s + 1) * P]

    s16 = np.zeros((P, _NS16), np.float16)
    s16[:, _W1_OFF:_W1_OFF + 1024] = w1.reshape(P, 1024).astype(np.float16)
    s16[:, _W2_OFF:_W2_OFF + 2048] = w2.reshape(P, 2048).astype(np.float16)
    s16[0:8, _B2_OFF:_B2_OFF + 256] = b2.reshape(8, 256).astype(np.float16)
    s16[:, _SAW_OFF:_SAW_OFF + 16] = saw.reshape(P, 16).astype(np.float16)

    r16 = np.zeros((1, _NR16), np.float16)
    r16[0, _CB_OFF:_CB_OFF + 512] = np.asarray(cov4_b, f32).astype(np.float16)
    r16[0, _B1_OFF:_B1_OFF + 512] = b1.reshape(512).astype(np.float16)
    r16[0, _SAB_OFF:_SAB_OFF + 1024] = np.repeat(
        np.asarray(sa_b, f32), 128).astype(np.float16)

    return w_arr, s16, r16


def kernel(**inputs):
    from concourse.bass_utils import run_bass_kernel_spmd

    if "nc" not in _CACHE:
        _CACHE["nc"] = _build_program()
    nc = _CACHE["nc"]

    x4 = np.asarray(inputs["x4"], np.float32)
    B = x4.shape[0]
    w_arr, s16, r16 = _prep_params(
        inputs["cov4_w"], inputs["cov4_b"],
        inputs["gce_fc1_w"], inputs["gce_fc1_b"],
        inputs["gce_fc2_w"], inputs["gce_fc2_b"],
        inputs["gce_sa_w"], inputs["gce_sa_b"])

    in_maps = [
        _prep_core_inputs(x4[b].reshape(2048, PX), w_arr, s16, r16)
        for b in range(B)
    ]
    res = run_bass_kernel_spmd(nc, in_maps, list(range(N_CORES)))
    _CACHE["last_results"] = res

    out = np.empty((B, 2048, 16, 16), np.float32)
    for b in range(B):
        o = res.results[b]["out"].astype(np.float32)   # [4, 128, 2, 2, 256]
        full = out[b].reshape(8, 2, P, PX)             # [g, s, part, px]
        for p in range(2):
            for pair in range(2):
                for ii in range(2):
                    g = p + 2 * (2 * pair + ii)
                    full[g] = o[2 * p + pair, :, ii, :, :].transpose(1, 0, 2)
    return out


# revision 33
# speedup vs baseline: 1.0642x; 1.0271x over previous
"""Trainium2 Bass kernel for nn_DSCAMSFF (1x1 conv + per-group CBAM gating).

Only x4 is live in the reference model (cov1-3 / the attention path are dead
code that returns its first argument). Effective computation per batch b:

  a  = conv1x1(x4[b]) : [512, 256]          (w [512,2048], pixels flattened)
  per group g (channels of group g are a[(g%2)*256 : (g%2+1)*256]):
    avg_g = mean_px(a_g)                       [256]
    h_g   = relu(fc1_w[g] @ avg_g + fc1_b[g])  [64]
    ca_g  = sigmoid(fc2_w[g] @ h_g + fc2_b[g]) [256]
    sa_g  = sigmoid((ca_g*sa_w[g]) . a_g + sa_b[g])   [256 px]
    z_g   = sigmoid(a_g * ca_g[:,None] * sa_g[None,:])
    mask  = where(z_g > mean(z_g), 1, z_g)
    out_g = a_g * (mask + 1)

Sharding: pure data-parallel over batch (8 cores x 1 batch element),
parameters replicated.

v2 schedule: input DMAs issued first (x, w0, w1, params, w2, w3) so conv
m-tiles start as weights land; the pixel-mean is folded into the conv as a
257th x column; output is written fp16 and upcast on host; the z-chain is
balanced across ACT (sigmoid w/ per-partition ca scale), DVE (u-mult, group
z-sums, fused mask-mul) and GpSimd (one u-mult per half).
"""

import numpy as np

N_CORES = 8
P = 128
PX = 256            # 16*16 pixels
PXA = 257           # pixels + folded mean column
KT = 16             # 2048 / 128 K tiles
MT = 4              # 512 / 128 conv out tiles

# s16 packed layout (columns)
_W1_OFF = 0         # [p, kt, m]    2*2*256  = 1024
_W2_OFF = 1024      # [p, i, s, m]  2*4*2*128 = 2048
_B2_OFF = 3072      # fc2 bias pack [8 rows, p*128 cols] = 256
_SAW_OFF = 3328     # [p, s, i] 16
_NS16 = 3344
# r16 row tensor (partition 0 only)
_CB_OFF = 0         # conv bias row [512]
_B1_OFF = 512       # fc1 bias row  [p, mt, 128] = 512
_SAB_OFF = 1024     # spatial bias row [8, 128] (replicated)
_NR16 = 2048

_CACHE = {}


def _register_dve_ops():
    """Register the fused mask-mul DVE op (idempotent, runtime-only)."""
    from concourse import dve_ops as DO
    from concourse.dve_spec import Spec, Src0, Src1, One, select

    if "DSCAM_MASK_MUL" in DO._SUB_OPCODE_FOR_NAME:
        return next(o for o in DO.OPS if o.name == "DSCAM_MASK_MUL")

    from concourse.dve_spec import C0, lower
    from concourse.dve_uop import DveOpSpec

    name = "DSCAM_MASK_MUL"
    row = DO._CUSTOM_DVE_ROW_BASE + len(DO.OPS)
    DO._SUB_OPCODE_FOR_NAME[name] = row
    spec = Spec(
        body=Src1 * (One + select(Src0 > C0, One, Src0)),
        reference=lambda in0, in1, s0, s1, imm2:
            (in1.astype(np.float32)
             * (1.0 + np.where(in0.astype(np.float32) > s0, 1.0,
                               in0.astype(np.float32)))).astype(np.float32),
    )
    shas = {}
    for ver in ("v3", "v4"):
        try:
            uops = lower(spec, ver=ver)
            shas[ver] = DveOpSpec(name=name, opcode=row, uops=uops,
                                  rd1_en=True).sha(ver)
        except Exception:
            pass
    op = DO.DveOp(name, spec, subdim=False, uops_sha=shas)
    DO.OPS.append(op)
    DO.CUSTOM_DVE_SPECS[name] = spec
    return op


def _build_program():
    import concourse.mybir as mybir
    import concourse.tile as tile
    from concourse import bacc

    fp32 = mybir.dt.float32
    fp16 = mybir.dt.float16
    Act = mybir.ActivationFunctionType
    Alu = mybir.AluOpType
    AX = mybir.AxisListType

    _MSK_OP = _register_dve_ops()

    nc = bacc.Bacc("TRN2", target_bir_lowering=False, debug=False)

    x_d = nc.dram_tensor("x", [P, KT, PXA], fp16, kind="ExternalInput").ap()
    fp8e3 = mybir.dt.float8e3
    w_d = nc.dram_tensor("w", [MT, P, KT, P], fp8e3,
                         kind="ExternalInput").ap()
    s16_d = nc.dram_tensor("s16", [P, _NS16], fp16, kind="ExternalInput").ap()
    r16_d = nc.dram_tensor("r16", [1, _NR16], fp16, kind="ExternalInput").ap()
    i8_d = nc.dram_tensor("i8", [8, 8], fp16, kind="ExternalInput").ap()
    # out chunk c = 2*p + pair holds groups (p + 2*(2*pair), p + 2*(2*pair+1))
    out_d = nc.dram_tensor("out", [4, P, 2, 2, PX], fp16,
                           kind="ExternalOutput").ap()

    with tile.TileContext(nc) as tc:
        with (
            tc.tile_pool(name="singles", bufs=1) as singles,
            tc.tile_pool(name="upool", bufs=4) as upool,
            tc.tile_pool(name="zpool", bufs=5) as zpool,
            tc.tile_pool(name="otp", bufs=3) as otp,
            tc.tile_pool(name="psC", bufs=2, space="PSUM") as psC,
            tc.tile_pool(name="psS", bufs=2, space="PSUM") as psS,
            tc.tile_pool(name="psT", bufs=2, space="PSUM") as psT,
        ):
            # ---- input DMAs first: ordered by need time ----
            xt = singles.tile([P, KT, PXA], fp16, tag="xt")
            wt = [singles.tile([P, KT, P], fp8e3, tag=f"w{m}", name=f"w{m}")
                  for m in range(MT)]
            s16 = singles.tile([P, _NS16], fp16, tag="s16")
            r16 = singles.tile([1, _NR16], fp16, tag="r16")
            id8 = singles.tile([8, 8], fp16, tag="id8")

            # x/w stream on the sync HWDGE ring (FIFO): w0 first, then x
            # in 4 K-chunks so conv m0 starts on partial x; params on the
            # scalar ring run concurrently
            nc.sync.dma_start(out=wt[0], in_=w_d[0])
            for c in range(4):
                nc.sync.dma_start(out=xt[:, 4 * c:4 * c + 4, :],
                                  in_=x_d[:, 4 * c:4 * c + 4, :])
            nc.sync.dma_start(out=wt[1], in_=w_d[1])
            nc.sync.dma_start(out=wt[2], in_=w_d[2])
            nc.sync.dma_start(out=wt[3], in_=w_d[3])
            nc.scalar.dma_start(out=s16, in_=s16_d)
            nc.scalar.dma_start(out=r16, in_=r16_d)
            nc.scalar.dma_start(out=id8, in_=i8_d)

            # parameter views
            w1v = s16[:, _W1_OFF:_W1_OFF + 1024].rearrange(
                "P (p k m) -> P p k m", p=2, k=2)
            w2v = s16[:, _W2_OFF:_W2_OFF + 2048].rearrange(
                "P (p i s m) -> P p i s m", p=2, i=4, s=2)
            b2v = s16[0:8, _B2_OFF:_B2_OFF + 256].rearrange(
                "o (p m) -> o p m", p=2)
            cbias = r16[:, _CB_OFF:_CB_OFF + 512].rearrange(
                "o (m c) -> o m c", m=4)
            b1row = r16[:, _B1_OFF:_B1_OFF + 512].rearrange(
                "o (p t c) -> o p t c", p=2, t=2)
            sab16 = r16[:, _SAB_OFF:_SAB_OFF + 1024].rearrange(
                "o (g c) -> o g c", g=8)
            sawv = s16[:, _SAW_OFF:_SAW_OFF + 16].rearrange(
                "P (p s i) -> P p s i", p=2, s=2)

            # constants (memsets overlap the DMA stream)
            ones16 = singles.tile([1, PXA], fp16, tag="ones16")
            nc.gpsimd.memset(ones16, 1.0)
            onesPK = singles.tile([P, PX], fp16, tag="onesPK")
            nc.gpsimd.memset(onesPK, 1.0)
            # ACT table preload; PE HAM warmup while input DMAs stream
            tl = singles.tile([1, 1], fp32, tag="tl")
            nc.scalar.activation(out=tl, in_=ones16[:, 0:1], func=Act.Sigmoid)
            for wu in range(18):
                wps = psS.tile([P, PX], fp32, tag="sa")
                nc.tensor.matmul(wps, lhsT=onesPK[:, 0:P], rhs=onesPK,
                                 start=True, stop=True)

            a16 = [None, None]    # per half: [P, 2, PXA] fp16 (col 256 = avg)
            h16 = [None, None]
            ca = [None, None]
            weff16 = [None, None]

            def conv_m(m):
                # one conv out tile -> psum [P, PXA]; col 256 = pixel mean
                ps = psC.tile([P, PXA], fp32, tag="conv")
                for kt in range(KT):
                    nc.tensor.matmul(ps, lhsT=wt[m][:, kt, :],
                                     rhs=xt[:, kt, :],
                                     start=(kt == 0), stop=False)
                nc.tensor.matmul(ps, lhsT=cbias[:, m, :], rhs=ones16,
                                 start=False, stop=True)
                p, s = m // 2, m % 2
                if a16[p] is None:
                    a16[p] = singles.tile([P, 2, PXA], fp16, tag=f"a16_{p}",
                                          name=f"a16_{p}")
                # PSUM -> SBUF fp16, undoing the 64x fp8 weight scale; the
                # ACT engine is idle during the first conv half, GpSimd later
                nc.scalar.activation(out=a16[p][:, s, :], in_=ps,
                                     func=Act.Copy, scale=1.0 / 64.0)

            def fc_chain(p):
                # fc1: h = relu(W1 @ avg + b1); avg = a16 col 256
                hp = psT.tile([P, 2], fp32, tag="tiny")
                for mt in (0, 1):
                    for kt in (0, 1):
                        nc.tensor.matmul(
                            hp[:, mt:mt + 1],
                            lhsT=w1v[:, p, kt, mt * P:(mt + 1) * P],
                            rhs=a16[p][:, kt, 256:257],
                            start=(kt == 0), stop=False)
                    nc.tensor.matmul(
                        hp[:, mt:mt + 1], lhsT=b1row[:, p, mt, :],
                        rhs=ones16[:, 0:1], start=False, stop=True)
                h16[p] = singles.tile([P, 2], fp16, tag=f"h{p}", name=f"h{p}")
                nc.scalar.activation(out=h16[p], in_=hp, func=Act.Relu)
                # fc2: ca = sigmoid(W2 @ h + b2). W2's unused 64 rows are
                # zero, so the stacked h (both groups of an mt) is safe as rhs.
                cp = psT.tile([P, 2, 4], fp32, tag="tiny")
                for s in (0, 1):
                    for i in range(4):
                        nc.tensor.matmul(
                            cp[:, s, i:i + 1], lhsT=w2v[:, p, i, s, :],
                            rhs=h16[p][:, i // 2:i // 2 + 1],
                            start=True, stop=False)
                        nc.tensor.matmul(
                            cp[:, s, i:i + 1], lhsT=b2v[:, p, :],
                            rhs=id8[:, 4 * s + i:4 * s + i + 1],
                            start=False, stop=True)
                ca[p] = singles.tile([P, 2, 4], fp32, tag=f"ca{p}",
                                     name=f"ca{p}")
                nc.scalar.activation(out=ca[p], in_=cp, func=Act.Sigmoid)
                weff16[p] = singles.tile([P, 2, 4], fp16, tag=f"we{p}",
                                         name=f"we{p}")
                nc.vector.tensor_tensor(out=weff16[p], in0=ca[p],
                                        in1=sawv[:, p], op=Alu.mult)

            def saz_mask(p):
                av = a16[p][:, :, 0:PX]
                sarep = [None, None]
                for j in range(2):
                    # groups g = p + 2*(2j), p + 2*(2j+1) share one psum bank
                    sps = psS.tile([P, 2, PX], fp32, tag="sa")
                    for jj in (0, 1):
                        i = 2 * j + jj
                        for s in (0, 1):
                            nc.tensor.matmul(
                                sps[:, jj, :],
                                lhsT=weff16[p][:, s, i:i + 1]
                                .to_broadcast((P, P)),
                                rhs=av[:, s, :],
                                start=(s == 0), stop=False)
                        nc.tensor.matmul(
                            sps[:, jj, :], lhsT=sab16[:, p + 2 * i, :],
                            rhs=ones16[:, 0:PX], start=False, stop=True)
                    sarep[j] = upool.tile([P, 2, PX], fp16, tag="sarep",
                                          name="sarep")
                    nc.scalar.activation(out=sarep[j], in_=sps,
                                         func=Act.Sigmoid)
                zsum = singles.tile([P, 4], fp16, tag=f"zs{p}", name=f"zs{p}")
                zs = [None, None]
                for i in range(4):
                    sr = sarep[i // 2][:, i % 2, :]
                    u16 = upool.tile([P, 2, PX], fp16, tag="u16")
                    # engine balance: DVE does 2 u-mults, GpSimd the other 2
                    eng = nc.gpsimd if i >= 2 else nc.vector
                    eng.tensor_tensor(
                        out=u16, in0=av,
                        in1=sr[:, None, :].to_broadcast((P, 2, PX)),
                        op=Alu.mult)
                    pair, ii = i // 2, i % 2
                    if ii == 0:
                        zs[pair] = zpool.tile([P, 2, 2, PX], fp16, tag="z",
                                              name="zpair")
                    z16 = zs[pair][:, ii]
                    for s in (0, 1):
                        nc.scalar.activation(
                            out=z16[:, s, :], in_=u16[:, s, :],
                            func=Act.Sigmoid, scale=ca[p][:, s, i:i + 1])
                    if ii == 1:
                        zr = psT.tile([P, 2], fp32, tag="tiny")
                        if p == 1 and pair == 1:
                            # PE-based group sums (DVE is the z-tail
                            # bottleneck): column sums via z-as-weights
                            # matmuls, then two more tiny MMs to total and
                            # broadcast
                            cs = psT.tile([P, 2], fp32, tag="tiny")
                            for jj in (0, 1):
                                zf = zs[pair][:, jj].rearrange(
                                    "P s f -> P (s f)")
                                for q in range(4):
                                    nc.tensor.matmul(
                                        cs[:, jj:jj + 1],
                                        lhsT=zf[:, q * P:(q + 1) * P],
                                        rhs=onesPK[:, 0:1],
                                        start=(q == 0), stop=(q == 3))
                            cs16 = singles.tile([P, 2], fp16, tag="cs16")
                            nc.vector.tensor_copy(out=cs16, in_=cs)
                            zrT = psT.tile([1, 2], fp32, tag="tiny")
                            nc.tensor.matmul(zrT, lhsT=onesPK[:, 0:1],
                                             rhs=cs16, start=True, stop=True)
                            t16 = singles.tile([1, 2], fp16, tag="t16")
                            nc.vector.tensor_copy(out=t16, in_=zrT)
                            for jj in (0, 1):
                                nc.tensor.matmul(
                                    zr[:, jj:jj + 1],
                                    lhsT=t16[:, jj:jj + 1]
                                    .to_broadcast((1, P)),
                                    rhs=ones16[:, 0:1],
                                    start=True, stop=True)
                        else:
                            # pair-wide per-partition z-sums on DVE, then the
                            # cross-partition sum via an all-ones fp16 matmul
                            with nc.allow_low_precision(
                                    reason="fp16 partial z-sums; ~1e-4"):
                                nc.vector.tensor_reduce(
                                    zsum[:, 2 * pair:2 * pair + 2],
                                    zs[pair].rearrange(
                                        "P i s f -> P i (s f)"),
                                    axis=AX.X, op=Alu.add)
                            nc.tensor.matmul(
                                zr, lhsT=onesPK[:, 0:P],
                                rhs=zsum[:, 2 * pair:2 * pair + 2],
                                start=True, stop=True)
                        pm = singles.tile([P, 2], fp32, tag=f"pm{p}{pair}",
                                          name=f"pm{p}{pair}")
                        nc.vector.tensor_scalar_mul(pm, zr, 1.0 / 65536.0)
                        ot = otp.tile([P, 2, 2, PX], fp16, tag="ot", name="ot")
                        for jj in (0, 1):
                            # fused out = a * (1 + where(z > mean, 1, z))
                            nc.vector._custom_dve(
                                _MSK_OP, out=ot[:, jj], in0=zs[pair][:, jj],
                                in1=av, s0=pm[:, jj:jj + 1])
                        nc.sync.dma_start(out=out_d[2 * p + pair], in_=ot)

            conv_m(0)
            conv_m(1)
            fc_chain(0)
            saz_mask(0)
            conv_m(2)
            conv_m(3)
            fc_chain(1)
            saz_mask(1)

    nc.finalize()
    return nc


def _prep_core_inputs(x4b, w, s16, r16):
    x = np.ascontiguousarray(
        x4b.reshape(KT, P, PX).transpose(1, 0, 2))
    xa = np.empty((P, KT, PXA), np.float16)
    xa[:, :, 0:PX] = x.astype(np.float16)
    xa[:, :, PX] = (x.mean(axis=2) * 1.0).astype(np.float16)
    return {"x": xa, "w": w, "s16": s16, "r16": r16,
            "i8": np.eye(8, dtype=np.float16)}


def _prep_params(cov4_w, cov4_b, fc1_w, fc1_b, fc2_w, fc2_b, sa_w, sa_b):
    import ml_dtypes
    f32 = np.float32
    w2d = np.asarray(cov4_w, f32).reshape(512, 2048)
    wr = w2d.reshape(MT, P, KT, P)                 # [m, mc, kt, part]
    # conv weights as fp8 e3m4, scaled by 64 (undone in the PSUM->a16 copy)
    w_arr = np.ascontiguousarray(
        wr.transpose(0, 3, 2, 1) * 64.0).astype(ml_dtypes.float8_e3m4)

    fc1_w = np.asarray(fc1_w, f32)
    fc1_b = np.asarray(fc1_b, f32)
    fc2_w = np.asarray(fc2_w, f32)
    fc2_b = np.asarray(fc2_b, f32)
    sa_w = np.asarray(sa_w, f32)
    sa_b = np.asarray(sa_b, f32)

    w1 = np.zeros((P, 2, 2, 256), f32)
    w2 = np.zeros((P, 2, 4, 2, P), f32)
    b2 = np.zeros((8, 2, P), f32)
    b1 = np.zeros((2, 2, P), f32)
    saw = np.zeros((P, 2, 2, 4), f32)
    for p in range(2):
        W1s = np.concatenate([fc1_w[p + 2 * i] for i in range(4)], axis=0)
        b1s = np.concatenate([fc1_b[p + 2 * i] for i in range(4)], axis=0)
        for kt in range(2):
            w1[:, p, kt, :] = W1s[:, kt * P:(kt + 1) * P].T
        b1[p, 0] = b1s[:P]
        b1[p, 1] = b1s[P:]
        for i in range(4):
            g = p + 2 * i
            lo = 64 * (i % 2)           # rows holding fc2 weights
            for s in range(2):
                w2[lo:lo + 64, p, i, s, :] = fc2_w[g][s * P:(s + 1) * P, :].T
                b2[4 * s + i, p, :] = fc2_b[g, s * P:(s + 1) * P]
                saw[:, p, s, i] = sa_w[g, s * P:(s + 1) * P]

    s16 = np.zeros((P, _NS16), np.float16)
    s16[:, _W1_OFF:_W1_OFF + 1024] = w1.reshape(P, 1024).astype(np.float16)
    s16[:, _W2_OFF:_W2_OFF + 2048] = w2.reshape(P, 2048).astype(np.float16)
    s16[0:8, _B2_OFF:_B2_OFF + 256] = b2.reshape(8, 256).astype(np.float16)
    s16[:, _SAW_OFF:_SAW_OFF + 16] = saw.reshape(P, 16).astype(np.float16)

    r16 = np.zeros((1, _NR16), np.float16)
    r16[0, _CB_OFF:_CB_OFF + 512] = (
        np.asarray(cov4_b, f32) * 64.0).astype(np.float16)
    r16[0, _B1_OFF:_B1_OFF + 512] = b1.reshape(512).astype(np.float16)
    r16[0, _SAB_OFF:_SAB_OFF + 1024] = np.repeat(
        np.asarray(sa_b, f32), 128).astype(np.float16)

    return w_arr, s16, r16


def kernel(**inputs):
    from concourse.bass_utils import run_bass_kernel_spmd

    if "nc" not in _CACHE:
        _CACHE["nc"] = _build_program()
    nc = _CACHE["nc"]

    x4 = np.asarray(inputs["x4"], np.float32)
    B = x4.shape[0]
    w_arr, s16, r16 = _prep_params(
        inputs["cov4_w"], inputs["cov4_b"],
        inputs["gce_fc1_w"], inputs["gce_fc1_b"],
        inputs["gce_fc2_w"], inputs["gce_fc2_b"],
        inputs["gce_sa_w"], inputs["gce_sa_b"])

    in_maps = [
        _prep_core_inputs(x4[b].reshape(2048, PX), w_arr, s16, r16)
        for b in range(B)
    ]
    res = run_bass_kernel_spmd(nc, in_maps, list(range(N_CORES)))
    _CACHE["last_results"] = res

    out = np.empty((B, 2048, 16, 16), np.float32)
    for b in range(B):
        o = res.results[b]["out"].astype(np.float32)   # [4, 128, 2, 2, 256]
        full = out[b].reshape(8, 2, P, PX)             # [g, s, part, px]
        for p in range(2):
            for pair in range(2):
                for ii in range(2):
                    g = p + 2 * (2 * pair + ii)
                    full[g] = o[2 * p + pair, :, ii, :, :].transpose(1, 0, 2)
    return out


# revision 34
# speedup vs baseline: 1.0935x; 1.0275x over previous
"""Trainium2 Bass kernel for nn_DSCAMSFF (1x1 conv + per-group CBAM gating).

Only x4 is live in the reference model (cov1-3 / the attention path are dead
code that returns its first argument). Effective computation per batch b:

  a  = conv1x1(x4[b]) : [512, 256]          (w [512,2048], pixels flattened)
  per group g (channels of group g are a[(g%2)*256 : (g%2+1)*256]):
    avg_g = mean_px(a_g)                       [256]
    h_g   = relu(fc1_w[g] @ avg_g + fc1_b[g])  [64]
    ca_g  = sigmoid(fc2_w[g] @ h_g + fc2_b[g]) [256]
    sa_g  = sigmoid((ca_g*sa_w[g]) . a_g + sa_b[g])   [256 px]
    z_g   = sigmoid(a_g * ca_g[:,None] * sa_g[None,:])
    mask  = where(z_g > mean(z_g), 1, z_g)
    out_g = a_g * (mask + 1)

Sharding: pure data-parallel over batch (8 cores x 1 batch element),
parameters replicated.

v2 schedule: input DMAs issued first (x, w0, w1, params, w2, w3) so conv
m-tiles start as weights land; the pixel-mean is folded into the conv as a
257th x column; output is written fp16 and upcast on host; the z-chain is
balanced across ACT (sigmoid w/ per-partition ca scale), DVE (u-mult, group
z-sums, fused mask-mul) and GpSimd (one u-mult per half).
"""

import numpy as np

N_CORES = 8
P = 128
PX = 256            # 16*16 pixels
PXA = 257           # pixels + folded mean column
KT = 16             # 2048 / 128 K tiles
MT = 4              # 512 / 128 conv out tiles

# s16 packed layout (columns)
_W1_OFF = 0         # [p, kt, m]    2*2*256  = 1024
_W2_OFF = 1024      # [p, i, s, m]  2*4*2*128 = 2048
_B2_OFF = 3072      # fc2 bias pack [8 rows, p*128 cols] = 256
_SAW_OFF = 3328     # [p, s, i] 16
_NS16 = 3344
# r16 row tensor (partition 0 only)
_CB_OFF = 0         # conv bias row [512]
_B1_OFF = 512       # fc1 bias row  [p, mt, 128] = 512
_SAB_OFF = 1024     # spatial bias row [8, 128] (replicated)
_NR16 = 2048

_CACHE = {}


def _register_dve_ops():
    """Register the fused mask-mul DVE op (idempotent, runtime-only)."""
    from concourse import dve_ops as DO
    from concourse.dve_spec import Spec, Src0, Src1, One, select

    if "DSCAM_MASK_MUL" in DO._SUB_OPCODE_FOR_NAME:
        return next(o for o in DO.OPS if o.name == "DSCAM_MASK_MUL")

    from concourse.dve_spec import C0, lower
    from concourse.dve_uop import DveOpSpec

    name = "DSCAM_MASK_MUL"
    row = DO._CUSTOM_DVE_ROW_BASE + len(DO.OPS)
    DO._SUB_OPCODE_FOR_NAME[name] = row
    spec = Spec(
        body=Src1 * (One + select(Src0 > C0, One, Src0)),
        reference=lambda in0, in1, s0, s1, imm2:
            (in1.astype(np.float32)
             * (1.0 + np.where(in0.astype(np.float32) > s0, 1.0,
                               in0.astype(np.float32)))).astype(np.float32),
    )
    shas = {}
    for ver in ("v3", "v4"):
        try:
            uops = lower(spec, ver=ver)
            shas[ver] = DveOpSpec(name=name, opcode=row, uops=uops,
                                  rd1_en=True).sha(ver)
        except Exception:
            pass
    op = DO.DveOp(name, spec, subdim=False, uops_sha=shas)
    DO.OPS.append(op)
    DO.CUSTOM_DVE_SPECS[name] = spec
    return op


def _build_program():
    import concourse.mybir as mybir
    import concourse.tile as tile
    from concourse import bacc

    fp32 = mybir.dt.float32
    fp16 = mybir.dt.float16
    Act = mybir.ActivationFunctionType
    Alu = mybir.AluOpType
    AX = mybir.AxisListType

    _MSK_OP = _register_dve_ops()

    nc = bacc.Bacc("TRN2", target_bir_lowering=False, debug=False)

    x_d = nc.dram_tensor("x", [P, KT, PXA], fp16, kind="ExternalInput").ap()
    fp8e3 = mybir.dt.float8e3
    w_d = nc.dram_tensor("w", [MT, P, KT, P], fp8e3,
                         kind="ExternalInput").ap()
    s16_d = nc.dram_tensor("s16", [P, _NS16], fp16, kind="ExternalInput").ap()
    r16_d = nc.dram_tensor("r16", [1, _NR16], fp16, kind="ExternalInput").ap()
    i8_d = nc.dram_tensor("i8", [8, 8], fp16, kind="ExternalInput").ap()
    # out chunk c = 2*p + pair holds groups (p + 2*(2*pair), p + 2*(2*pair+1))
    out_d = nc.dram_tensor("out", [4, P, 2, 2, PX], fp16,
                           kind="ExternalOutput").ap()

    with tile.TileContext(nc) as tc:
        with (
            tc.tile_pool(name="singles", bufs=1) as singles,
            tc.tile_pool(name="upool", bufs=4) as upool,
            tc.tile_pool(name="zpool", bufs=5) as zpool,
            tc.tile_pool(name="otp", bufs=3) as otp,
            tc.tile_pool(name="psC", bufs=2, space="PSUM") as psC,
            tc.tile_pool(name="psS", bufs=2, space="PSUM") as psS,
            tc.tile_pool(name="psT", bufs=2, space="PSUM") as psT,
        ):
            # ---- input DMAs first: ordered by need time ----
            xt = singles.tile([P, KT, PXA], fp16, tag="xt")
            wt = [singles.tile([P, KT, P], fp8e3, tag=f"w{m}", name=f"w{m}")
                  for m in range(MT)]
            s16 = singles.tile([P, _NS16], fp16, tag="s16")
            r16 = singles.tile([1, _NR16], fp16, tag="r16")
            id8 = singles.tile([8, 8], fp16, tag="id8")

            # x/w stream on the sync HWDGE ring (FIFO): w0 first, then x
            # in 4 K-chunks so conv m0 starts on partial x; params on the
            # scalar ring run concurrently
            nc.sync.dma_start(out=wt[0], in_=w_d[0])
            for c in range(4):
                nc.sync.dma_start(out=xt[:, 4 * c:4 * c + 4, :],
                                  in_=x_d[:, 4 * c:4 * c + 4, :])
            nc.sync.dma_start(out=wt[1], in_=w_d[1])
            nc.sync.dma_start(out=s16, in_=s16_d)
            nc.sync.dma_start(out=r16, in_=r16_d)
            nc.sync.dma_start(out=id8, in_=i8_d)
            nc.sync.dma_start(out=wt[2], in_=w_d[2])
            nc.sync.dma_start(out=wt[3], in_=w_d[3])

            # parameter views
            w1v = s16[:, _W1_OFF:_W1_OFF + 1024].rearrange(
                "P (p k m) -> P p k m", p=2, k=2)
            w2v = s16[:, _W2_OFF:_W2_OFF + 2048].rearrange(
                "P (p i s m) -> P p i s m", p=2, i=4, s=2)
            b2v = s16[0:8, _B2_OFF:_B2_OFF + 256].rearrange(
                "o (p m) -> o p m", p=2)
            cbias = r16[:, _CB_OFF:_CB_OFF + 512].rearrange(
                "o (m c) -> o m c", m=4)
            b1row = r16[:, _B1_OFF:_B1_OFF + 512].rearrange(
                "o (p t c) -> o p t c", p=2, t=2)
            sab16 = r16[:, _SAB_OFF:_SAB_OFF + 1024].rearrange(
                "o (g c) -> o g c", g=8)
            sawv = s16[:, _SAW_OFF:_SAW_OFF + 16].rearrange(
                "P (p s i) -> P p s i", p=2, s=2)

            # constants (memsets overlap the DMA stream)
            ones16 = singles.tile([1, PXA], fp16, tag="ones16")
            nc.gpsimd.memset(ones16, 1.0)
            onesPK = singles.tile([P, PX], fp16, tag="onesPK")
            nc.gpsimd.memset(onesPK, 1.0)
            # ACT table preload; PE HAM warmup while input DMAs stream
            tl = singles.tile([1, 1], fp32, tag="tl")
            nc.scalar.activation(out=tl, in_=ones16[:, 0:1], func=Act.Sigmoid)
            for wu in range(18):
                wps = psS.tile([P, PX], fp32, tag="sa")
                nc.tensor.matmul(wps, lhsT=onesPK[:, 0:P], rhs=onesPK,
                                 start=True, stop=True)

            a16 = [None, None]    # per half: [P, 2, PXA] fp16 (col 256 = avg)
            h16 = [None, None]
            ca = [None, None]
            weff16 = [None, None]

            def conv_m(m):
                # one conv out tile -> psum [P, PXA]; col 256 = pixel mean
                ps = psC.tile([P, PXA], fp32, tag="conv")
                for kt in range(KT):
                    nc.tensor.matmul(ps, lhsT=wt[m][:, kt, :],
                                     rhs=xt[:, kt, :],
                                     start=(kt == 0), stop=False)
                nc.tensor.matmul(ps, lhsT=cbias[:, m, :], rhs=ones16,
                                 start=False, stop=True)
                p, s = m // 2, m % 2
                if a16[p] is None:
                    a16[p] = singles.tile([P, 2, PXA], fp16, tag=f"a16_{p}",
                                          name=f"a16_{p}")
                # PSUM -> SBUF fp16, undoing the 64x fp8 weight scale; the
                # ACT engine is idle during the first conv half, GpSimd later
                nc.scalar.activation(out=a16[p][:, s, :], in_=ps,
                                     func=Act.Copy, scale=1.0 / 64.0)

            def fc_chain(p):
                # fc1: h = relu(W1 @ avg + b1); avg = a16 col 256
                hp = psT.tile([P, 2], fp32, tag="tiny")
                for mt in (0, 1):
                    for kt in (0, 1):
                        nc.tensor.matmul(
                            hp[:, mt:mt + 1],
                            lhsT=w1v[:, p, kt, mt * P:(mt + 1) * P],
                            rhs=a16[p][:, kt, 256:257],
                            start=(kt == 0), stop=False)
                    nc.tensor.matmul(
                        hp[:, mt:mt + 1], lhsT=b1row[:, p, mt, :],
                        rhs=ones16[:, 0:1], start=False, stop=True)
                h16[p] = singles.tile([P, 2], fp16, tag=f"h{p}", name=f"h{p}")
                nc.scalar.activation(out=h16[p], in_=hp, func=Act.Relu)
                # fc2: ca = sigmoid(W2 @ h + b2). W2's unused 64 rows are
                # zero, so the stacked h (both groups of an mt) is safe as rhs.
                cp = psT.tile([P, 2, 4], fp32, tag="tiny")
                for s in (0, 1):
                    for i in range(4):
                        nc.tensor.matmul(
                            cp[:, s, i:i + 1], lhsT=w2v[:, p, i, s, :],
                            rhs=h16[p][:, i // 2:i // 2 + 1],
                            start=True, stop=False)
                        nc.tensor.matmul(
                            cp[:, s, i:i + 1], lhsT=b2v[:, p, :],
                            rhs=id8[:, 4 * s + i:4 * s + i + 1],
                            start=False, stop=True)
                ca[p] = singles.tile([P, 2, 4], fp32, tag=f"ca{p}",
                                     name=f"ca{p}")
                nc.scalar.activation(out=ca[p], in_=cp, func=Act.Sigmoid)
                weff16[p] = singles.tile([P, 2, 4], fp16, tag=f"we{p}",
                                         name=f"we{p}")
                nc.vector.tensor_tensor(out=weff16[p], in0=ca[p],
                                        in1=sawv[:, p], op=Alu.mult)

            def saz_mask(p):
                av = a16[p][:, :, 0:PX]
                sarep = [None, None]
                for j in range(2):
                    # groups g = p + 2*(2j), p + 2*(2j+1) share one psum bank
                    sps = psS.tile([P, 2, PX], fp32, tag="sa")
                    for jj in (0, 1):
                        i = 2 * j + jj
                        for s in (0, 1):
                            nc.tensor.matmul(
                                sps[:, jj, :],
                                lhsT=weff16[p][:, s, i:i + 1]
                                .to_broadcast((P, P)),
                                rhs=av[:, s, :],
                                start=(s == 0), stop=False)
                        nc.tensor.matmul(
                            sps[:, jj, :], lhsT=sab16[:, p + 2 * i, :],
                            rhs=ones16[:, 0:PX], start=False, stop=True)
                    sarep[j] = upool.tile([P, 2, PX], fp16, tag="sarep",
                                          name="sarep")
                    nc.scalar.activation(out=sarep[j], in_=sps,
                                         func=Act.Sigmoid)
                zsum = singles.tile([P, 4], fp16, tag=f"zs{p}", name=f"zs{p}")
                zs = [None, None]
                for i in range(4):
                    sr = sarep[i // 2][:, i % 2, :]
                    u16 = upool.tile([P, 2, PX], fp16, tag="u16")
                    # engine balance: DVE does 2 u-mults, GpSimd the other 2
                    eng = nc.gpsimd if i >= 2 else nc.vector
                    eng.tensor_tensor(
                        out=u16, in0=av,
                        in1=sr[:, None, :].to_broadcast((P, 2, PX)),
                        op=Alu.mult)
                    pair, ii = i // 2, i % 2
                    if ii == 0:
                        zs[pair] = zpool.tile([P, 2, 2, PX], fp16, tag="z",
                                              name="zpair")
                    z16 = zs[pair][:, ii]
                    for s in (0, 1):
                        nc.scalar.activation(
                            out=z16[:, s, :], in_=u16[:, s, :],
                            func=Act.Sigmoid, scale=ca[p][:, s, i:i + 1])
                    if ii == 1:
                        zr = psT.tile([P, 2], fp32, tag="tiny")
                        if p == 1 and pair == 0:
                            # PE-based group sums (DVE is the z-tail
                            # bottleneck): column sums via z-as-weights
                            # matmuls, then two more tiny MMs to total and
                            # broadcast
                            cs = psT.tile([P, 2], fp32, tag="tiny")
                            for jj in (0, 1):
                                zf = zs[pair][:, jj].rearrange(
                                    "P s f -> P (s f)")
                                for q in range(4):
                                    nc.tensor.matmul(
                                        cs[:, jj:jj + 1],
                                        lhsT=zf[:, q * P:(q + 1) * P],
                                        rhs=onesPK[:, 0:1],
                                        start=(q == 0), stop=(q == 3))
                            cs16 = singles.tile([P, 2], fp16, tag="cs16")
                            nc.vector.tensor_copy(out=cs16, in_=cs)
                            zrT = psT.tile([1, 2], fp32, tag="tiny")
                            nc.tensor.matmul(zrT, lhsT=onesPK[:, 0:1],
                                             rhs=cs16, start=True, stop=True)
                            t16 = singles.tile([1, 2], fp16, tag="t16")
                            nc.vector.tensor_copy(out=t16, in_=zrT)
                            for jj in (0, 1):
                                nc.tensor.matmul(
                                    zr[:, jj:jj + 1],
                                    lhsT=t16[:, jj:jj + 1]
                                    .to_broadcast((1, P)),
                                    rhs=ones16[:, 0:1],
                                    start=True, stop=True)
                        else:
                            # pair-wide per-partition z-sums on DVE, then the
                            # cross-partition sum via an all-ones fp16 matmul
                            with nc.allow_low_precision(
                                    reason="fp16 partial z-sums; ~1e-4"):
                                nc.vector.tensor_reduce(
                                    zsum[:, 2 * pair:2 * pair + 2],
                                    zs[pair].rearrange(
                                        "P i s f -> P i (s f)"),
                                    axis=AX.X, op=Alu.add)
                            nc.tensor.matmul(
                                zr, lhsT=onesPK[:, 0:P],
                                rhs=zsum[:, 2 * pair:2 * pair + 2],
                                start=True, stop=True)
                        pm = singles.tile([P, 2], fp32, tag=f"pm{p}{pair}",
                                          name=f"pm{p}{pair}")
                        nc.vector.tensor_scalar_mul(pm, zr, 1.0 / 65536.0)
                        ot = otp.tile([P, 2, 2, PX], fp16, tag="ot", name="ot")
                        for jj in (0, 1):
                            # fused out = a * (1 + where(z > mean, 1, z))
                            nc.vector._custom_dve(
                                _MSK_OP, out=ot[:, jj], in0=zs[pair][:, jj],
                                in1=av, s0=pm[:, jj:jj + 1])
                        nc.sync.dma_start(out=out_d[2 * p + pair], in_=ot)

            conv_m(0)
            conv_m(1)
            fc_chain(0)
            saz_mask(0)
            conv_m(2)
            conv_m(3)
            fc_chain(1)
            saz_mask(1)

    nc.finalize()
    return nc


def _prep_core_inputs(x4b, w, s16, r16):
    x = np.ascontiguousarray(
        x4b.reshape(KT, P, PX).transpose(1, 0, 2))
    xa = np.empty((P, KT, PXA), np.float16)
    xa[:, :, 0:PX] = x.astype(np.float16)
    xa[:, :, PX] = (x.mean(axis=2) * 1.0).astype(np.float16)
    return {"x": xa, "w": w, "s16": s16, "r16": r16,
            "i8": np.eye(8, dtype=np.float16)}


def _prep_params(cov4_w, cov4_b, fc1_w, fc1_b, fc2_w, fc2_b, sa_w, sa_b):
    import ml_dtypes
    f32 = np.float32
    w2d = np.asarray(cov4_w, f32).reshape(512, 2048)
    wr = w2d.reshape(MT, P, KT, P)                 # [m, mc, kt, part]
    # conv weights as fp8 e3m4, scaled by 64 (undone in the PSUM->a16 copy)
    w_arr = np.ascontiguousarray(
        wr.transpose(0, 3, 2, 1) * 64.0).astype(ml_dtypes.float8_e3m4)

    fc1_w = np.asarray(fc1_w, f32)
    fc1_b = np.asarray(fc1_b, f32)
    fc2_w = np.asarray(fc2_w, f32)
    fc2_b = np.asarray(fc2_b, f32)
    sa_w = np.asarray(sa_w, f32)
    sa_b = np.asarray(sa_b, f32)

    w1 = np.zeros((P, 2, 2, 256), f32)
    w2 = np.zeros((P, 2, 4, 2, P), f32)
    b2 = np.zeros((8, 2, P), f32)
    b1 = np.zeros((2, 2, P), f32)
    saw = np.zeros((P, 2, 2, 4), f32)
    for p in range(2):
        W1s = np.concatenate([fc1_w[p + 2 * i] for i in range(4)], axis=0)
        b1s = np.concatenate([fc1_b[p + 2 * i] for i in range(4)], axis=0)
        for kt in range(2):
            w1[:, p, kt, :] = W1s[:, kt * P:(kt + 1) * P].T
        b1[p, 0] = b1s[:P]
        b1[p, 1] = b1s[P:]
        for i in range(4):
            g = p + 2 * i
            lo = 64 * (i % 2)           # rows holding fc2 weights
            for s in range(2):
                w2[lo:lo + 64, p, i, s, :] = fc2_w[g][s * P:(s + 1) * P, :].T
                b2[4 * s + i, p, :] = fc2_b[g, s * P:(s + 1) * P]
                saw[:, p, s, i] = sa_w[g, s * P:(s + 1) * P]

    s16 = np.zeros((P, _NS16), np.float16)
    s16[:, _W1_OFF:_W1_OFF + 1024] = w1.reshape(P, 1024).astype(np.float16)
    s16[:, _W2_OFF:_W2_OFF + 2048] = w2.reshape(P, 2048).astype(np.float16)
    s16[0:8, _B2_OFF:_B2_OFF + 256] = b2.reshape(8, 256).astype(np.float16)
    s16[:, _SAW_OFF:_SAW_OFF + 16] = saw.reshape(P, 16).astype(np.float16)

    r16 = np.zeros((1, _NR16), np.float16)
    r16[0, _CB_OFF:_CB_OFF + 512] = (
        np.asarray(cov4_b, f32) * 64.0).astype(np.float16)
    r16[0, _B1_OFF:_B1_OFF + 512] = b1.reshape(512).astype(np.float16)
    r16[0, _SAB_OFF:_SAB_OFF + 1024] = np.repeat(
        np.asarray(sa_b, f32), 128).astype(np.float16)

    return w_arr, s16, r16


def kernel(**inputs):
    from concourse.bass_utils import run_bass_kernel_spmd

    if "nc" not in _CACHE:
        _CACHE["nc"] = _build_program()
    nc = _CACHE["nc"]

    x4 = np.asarray(inputs["x4"], np.float32)
    B = x4.shape[0]
    w_arr, s16, r16 = _prep_params(
        inputs["cov4_w"], inputs["cov4_b"],
        inputs["gce_fc1_w"], inputs["gce_fc1_b"],
        inputs["gce_fc2_w"], inputs["gce_fc2_b"],
        inputs["gce_sa_w"], inputs["gce_sa_b"])

    in_maps = [
        _prep_core_inputs(x4[b].reshape(2048, PX), w_arr, s16, r16)
        for b in range(B)
    ]
    res = run_bass_kernel_spmd(nc, in_maps, list(range(N_CORES)))
    _CACHE["last_results"] = res

    out = np.empty((B, 2048, 16, 16), np.float32)
    for b in range(B):
        o = res.results[b]["out"].astype(np.float32)   # [4, 128, 2, 2, 256]
        full = out[b].reshape(8, 2, P, PX)             # [g, s, part, px]
        for p in range(2):
            for pair in range(2):
                for ii in range(2):
                    g = p + 2 * (2 * pair + ii)
                    full[g] = o[2 * p + pair, :, ii, :, :].transpose(1, 0, 2)
    return out
